# revision 25
# baseline (speedup 1.0000x reference)
"""Distributed BasicGCN kernel for one Trainium2 chip (8 NeuronCores).

Strategy (graph/data parallel, hardcoded for N=50000 nodes / E=800000 edges):
  - Nodes are partitioned contiguously across the 8 cores (6250 each, padded
    to 6272 = 49*128). Node features h live on-chip in feature-major layout
    (hT: [128 feat x 2 halves, 6272 node cols], fp16).
  - Per GCN layer, each core computes g = (h @ W) * dinv for its node shard
    (TensorE), emits it fp8 node-major to g_local, and a 4-way CHUNKED
    AllGather replicates it into a single fp8 table g_full [50176, 256].
    The dense matmuls for layer l+1's table are interleaved per-tile into
    layer l's scatter loop, so the chunked AllGathers overlap with the
    gather/scatter stream of the previous layer.
  - Edges are partitioned by destination. The weighted scatter-sum
    agg[d] = sum_e w_e * g[src_e] is computed as one-hot matmuls: gather 128
    source rows (SWDGE dma_gather, 4 queues) into an SBUF tile [128 edges,
    256 feat] fp8, then PE-matmul with a host-precomputed one-hot weight
    matrix [128 edges, 64 dst] (fp8, SBUF-resident for all layers)
    accumulating into PSUM [128 feat, 64 dst].
  - int16 gather indices cap a stream at 32768 rows, so edges are split into
    stream A (table rows [0, 32768)) and stream B (rows [17408, 50176));
    sources in the overlap are assigned to balance the two streams. A-calls
    go to SWDGE queues 0/1, B-calls to 2/3 (no head-of-line blocking while
    the last AllGather chunk, which only stream B needs, is in flight).
  - Self-loops are folded in as ordinary edges with weight dinv[d]; the
    symmetric norm dinv[s]*ew*dinv[d] is split as (g absorbs dinv[s],
    one-hot weight absorbs ew*dinv[d]).
  - Embedding and decode layers are plain sharded matmuls (fp16).

All host-side preprocessing (degree/norm computation, edge binning, one-hot
construction) is numpy; the device program structure is identical across
cores (required by SPMD), with per-core data shipped via in_maps.
"""

import sys

sys.path.insert(0, "/opt/trn_rl_repo")

import numpy as np
from ml_dtypes import float8_e4m3

# ---------------------------------------------------------------- constants
NC = 8
N_NODES = 50000
IN_FEAT = 7
INPUT_SIZE = 12
DIN = IN_FEAT * INPUT_SIZE  # 84
HID = 256
OUT_FEAT = 7
FH = 24
DOUT = OUT_FEAT * FH  # 168
N_LAYERS = 5

PER = N_NODES // NC  # 6250 real nodes per core
NT128 = 49  # node tiles of 128 per core
PERP = NT128 * 128  # 6272 padded nodes per core
D = 64  # destination-tile size for the scatter matmul
NT64 = PERP // D  # 98 dst tiles per core

# AllGather chunk geometry: 2 chunks of dense tiles per core (a Shared DRAM
# tensor may only have a single writer instruction, so one AG per stream)
CH_LR = [0, 3072, 6272]  # local row boundaries (tiles 24 / 25)
AGJ = {23: 0, 48: 1}  # dense tile j -> chunk issued after it
TROWS = PERP * NC  # 50176 table rows
BBASE = 24576  # stream B's base row: B covers [24576, 50176)
AROWS = 24576
BROWS = TROWS - BBASE  # 25600

MAXCH = 16  # 128-edge chunks per dma_gather call

ABLATE = set()  # dev-only: {"gather", "scatter_mm", "ag"}

F16 = np.float16
F32 = np.float32
F8 = float8_e4m3


def _cdiv(a, b):
    return -(-a // b)


# ------------------------------------------------------------ host prep
def _prep(edge_index, edge_weights):
    """Bin edges by (core, dst-tile, gather-stream), build per-core index and
    one-hot tensors plus the (uniform) program structure."""
    src = np.asarray(edge_index[0], dtype=np.int64)
    dst = np.asarray(edge_index[1], dtype=np.int64)
    ew = np.asarray(edge_weights, dtype=F32)

    deg = np.bincount(dst, weights=ew.astype(np.float64), minlength=N_NODES).astype(
        F32
    ) + F32(1.0)
    dinv = (1.0 / np.sqrt(deg)).astype(F32)

    # edges + self loops; one-hot weight = ew * dinv[dst] (self: dinv[d])
    allsrc = np.concatenate([src, np.arange(N_NODES, dtype=np.int64)])
    alldst = np.concatenate([dst, np.arange(N_NODES, dtype=np.int64)])
    allw = np.concatenate([ew * dinv[dst], dinv]).astype(F32)

    core_d = alldst // PER
    dl = alldst % PER
    t98 = dl // D
    gt = core_d * NT64 + t98  # global tile id
    dloc = dl % D

    # source -> table row (chunk-major AllGather layout)
    core_s = allsrc // PER
    sl = allsrc % PER
    kch = (sl >= CH_LR[1]).astype(np.int64)
    lr = np.asarray(CH_LR[:2], dtype=np.int64)
    sz = np.asarray([CH_LR[i + 1] - CH_LR[i] for i in range(2)], dtype=np.int64)
    grow = 8 * lr[kch] + core_s * sz[kch] + (sl - lr[kch])
    arow = grow  # valid iff grow < AROWS
    brow = grow - BBASE  # valid iff grow >= BBASE
    cls = np.where(grow < BBASE, 0, 2).astype(np.int64)

    NT = NC * NT64
    cnt = np.bincount(gt * 3 + cls, minlength=NT * 3).reshape(NC, NT64, 3)
    nAf, nfl, nBf = cnt[..., 0], cnt[..., 1], cnt[..., 2]
    ntot = nAf + nfl + nBf
    # per-tile chunk counts (uniform across cores); balance A/B globally
    ctot = _cdiv(ntot, 128).max(0)
    SAmin = _cdiv(nAf, 128).max(0)
    SBmin = _cdiv(nBf, 128).max(0)
    ctot = np.maximum(ctot, SAmin + SBmin)
    cA = np.clip(ctot // 2, SAmin, ctot - SBmin)
    cB = ctot - cA
    assert (cA >= 1).all() and (cB >= 1).all()  # both passes touch every tile
    SA = cA * 128
    nA = np.minimum(nAf + nfl, SA[None, :])  # per (core, tile) edges routed to A
    nB = ntot - nA
    assert (nB <= cB[None, :] * 128).all()

    AoffC = np.concatenate([[0], np.cumsum(cA)])  # chunk offset of tile t in A stream
    BoffC = np.concatenate([[0], np.cumsum(cB)])
    NAc = int(AoffC[-1])
    NBc = int(BoffC[-1])
    gb = np.concatenate([[0], np.cumsum(cA + cB)])  # group base per tile
    Gtot = int(gb[-1])

    # order edges by (tile, class); random tertiary key (sorted sources
    # hotspot the DMA engines on gather); A-selected = first nA of each seg
    shkey = np.random.default_rng(12345).permutation(len(allsrc))
    order = np.lexsort((shkey, cls, gt))
    gts = gt[order]
    seg_start = np.searchsorted(gts, np.arange(NT))
    r = np.arange(len(order)) - seg_start[gts]  # rank within tile segment
    oc = gts // NT64
    ot = gts % NT64
    isA = r < nA[oc, ot]
    rb = r - nA[oc, ot]

    osrcA = arow[order]
    osrcB = brow[order]
    ow = allw[order]
    odloc = dloc[order]

    # stream positions
    posA = AoffC[ot] * 128 + r  # valid where isA
    posB = BoffC[ot] * 128 + rb  # valid where ~isA
    grp = np.where(isA, gb[ot] + r // 128, gb[ot] + cA[ot] + rb // 128)
    pos128 = np.where(isA, r % 128, rb % 128)

    idxA = np.zeros((NC, max(NAc, 1) * 128), np.int16)
    idxB = np.zeros((NC, max(NBc, 1) * 128), np.int16)
    a = isA
    b = ~isA
    idxA[oc[a], posA[a]] = osrcA[a].astype(np.int16)
    idxB[oc[b], posB[b]] = osrcB[b].astype(np.int16)

    oh = np.zeros((NC, 128, Gtot * D), F8)
    oh[oc, pos128, grp * D + odloc] = ow.astype(F8)

    # sanity: all used indices in range
    assert osrcA[a].max(initial=0) < AROWS and osrcA[a].min(initial=0) >= 0
    assert osrcB[b].max(initial=0) < BROWS and osrcB[b].min(initial=0) >= 0

    def wrap(idx, nchunks):
        # [NC, n*128] -> [NC, 128, n*8]: position i lives at (i%16 + 16k, i//16)
        w16 = idx.reshape(NC, nchunks * 8, 16).transpose(0, 2, 1)
        return np.ascontiguousarray(np.tile(w16, (1, 8, 1)))

    idxA_w = wrap(idxA, max(NAc, 1))
    idxB_w = wrap(idxB, max(NBc, 1))

    # per-core dinv, padded
    dinv_pad = np.zeros((NC, PERP), F32)
    dinv_pad[:, :PER] = dinv.reshape(NC, PER)
    dinv_sb = np.ascontiguousarray(dinv_pad.reshape(NC, NT128, 128).transpose(0, 2, 1))

    return dict(
        dinv=dinv,
        cA=cA,
        cB=cB,
        AoffC=AoffC,
        BoffC=BoffC,
        NAc=NAc,
        NBc=NBc,
        gb=gb,
        Gtot=Gtot,
        idxA_w=idxA_w,
        idxB_w=idxB_w,
        oh=oh,
        dinv_sb=dinv_sb,
        idxA_flat=idxA,
        idxB_flat=idxB,
    )


def _pack_weights(W_emb, b_emb, Wg, bg, W_dec, b_dec):
    """Pack weight matrices into the on-device layouts (shared across cores)."""
    Wemb_p = np.ascontiguousarray(W_emb.astype(F16))  # [84, 256]
    bemb_p = np.ascontiguousarray(
        b_emb.astype(F32).reshape(2, 128).T
    )  # [128, 2] (col h = bias[h*128+p])
    # Wg packed [128, 5*2*256]: layer l slab k -> cols [(l*2+k)*256, ...)
    Wg_p = np.zeros((128, N_LAYERS * 2 * HID), F16)
    for l in range(N_LAYERS):
        for k in range(2):
            Wg_p[:, (l * 2 + k) * HID : (l * 2 + k + 1) * HID] = Wg[l][
                k * 128 : (k + 1) * 128, :
            ].astype(F16)
    bg_p = np.ascontiguousarray(
        bg.astype(F32).reshape(N_LAYERS, 2, 128).transpose(2, 0, 1).reshape(128, -1)
    )  # [128, 5*2]: col l*2+h
    Wdec_p = np.zeros((128, 2 * DOUT), F16)
    for k in range(2):
        Wdec_p[:, k * DOUT : (k + 1) * DOUT] = W_dec[k * 128 : (k + 1) * 128, :].astype(
            F16
        )
    bdec_p = np.ascontiguousarray(np.broadcast_to(b_dec.astype(F32), (128, DOUT)))
    return Wemb_p, bemb_p, Wg_p, bg_p, Wdec_p, bdec_p


def _prep_x(x):
    """x [50000, 7, 12] -> per-core xT [84, 6272] fp16 (feature-major)."""
    xf = x.reshape(N_NODES, DIN).astype(F16)
    xT = np.zeros((NC, DIN, PERP), F16)
    for c in range(NC):
        xT[c, :, :PER] = xf[c * PER : (c + 1) * PER].T
    return xT


# ------------------------------------------------------------ device build
def _build(st):
    import concourse.bass as bass
    import concourse.mybir as mybir
    from concourse import tile, bacc

    f16 = mybir.dt.float16
    f32 = mybir.dt.float32
    f8 = mybir.dt.float8e4
    i16 = mybir.dt.int16
    Relu = mybir.ActivationFunctionType.Relu
    core_ids = list(range(NC))

    cA, cB = st["cA"], st["cB"]
    AoffC, BoffC = st["AoffC"], st["BoffC"]
    NAc, NBc, gb = st["NAc"], st["NBc"], st["gb"]
    Gtot = st["Gtot"]

    nc = bacc.Bacc(
        "TRN2",
        target_bir_lowering=False,
        debug=False,
        num_devices=NC,
        num_swdge_queues=4,
    )

    xT_d = nc.dram_tensor("xT", [DIN, PERP], f16, kind="ExternalInput")
    dinv_d = nc.dram_tensor("dinv", [128, NT128], f32, kind="ExternalInput")
    Wemb_d = nc.dram_tensor("Wemb", [DIN, HID], f16, kind="ExternalInput")
    bemb_d = nc.dram_tensor("bemb", [128, 2], f32, kind="ExternalInput")
    Wg_d = nc.dram_tensor("Wg", [128, N_LAYERS * 2 * HID], f16, kind="ExternalInput")
    bg_d = nc.dram_tensor("bg", [128, N_LAYERS * 2], f32, kind="ExternalInput")
    Wdec_d = nc.dram_tensor("Wdec", [128, 2 * DOUT], f16, kind="ExternalInput")
    bdec_d = nc.dram_tensor("bdec", [128, DOUT], f32, kind="ExternalInput")
    idxA_d = nc.dram_tensor("idxA", [128, max(NAc, 1) * 8], i16, kind="ExternalInput")
    idxB_d = nc.dram_tensor("idxB", [128, max(NBc, 1) * 8], i16, kind="ExternalInput")
    oh_d = nc.dram_tensor("oh", [128, Gtot * D], f8, kind="ExternalInput")
    out_d = nc.dram_tensor("out", [PERP, DOUT], f32, kind="ExternalOutput")

    with tile.TileContext(nc) as tc:
        with (
            tc.tile_pool(name="const", bufs=1) as constp,
            tc.tile_pool(name="hbuf", bufs=1) as hbuf,
            tc.tile_pool(name="msgA", bufs=4) as msgAp,
            tc.tile_pool(name="msgB", bufs=4) as msgBp,
            tc.tile_pool(name="gstg", bufs=2) as gstg,
            tc.tile_pool(name="xstg", bufs=2) as xstg,
            tc.tile_pool(name="ostg", bufs=2) as ostg,
            tc.tile_pool(name="psg", bufs=2, space="PSUM") as psg,
            tc.tile_pool(name="pss", bufs=3, space="PSUM") as pss,
            tc.tile_pool(name="dram", bufs=1, space="DRAM") as dramp,
            tc.tile_pool(name="dramgA", bufs=2, space="DRAM") as dramgAp,
            tc.tile_pool(name="dramgB", bufs=2, space="DRAM") as dramgBp,
        ):
            # ---- persistent DRAM tiles
            g_local = dramp.tile([PERP, HID], f8, tag="g_local")

            # ---- constants into SBUF
            def load_const(dram_t, shape, dtype, tag):
                t = constp.tile(shape, dtype, tag=tag)
                nc.sync.dma_start(t[:], dram_t[:])
                return t

            dinv_sb = load_const(dinv_d, [128, NT128], f32, "dinv")
            Wemb_sb = load_const(Wemb_d, [DIN, HID], f16, "Wemb")
            bemb_sb = load_const(bemb_d, [128, 2], f32, "bemb")
            Wg_sb = load_const(Wg_d, [128, N_LAYERS * 2 * HID], f16, "Wg")
            bg_sb = load_const(bg_d, [128, N_LAYERS * 2], f32, "bg")
            Wdec_sb = load_const(Wdec_d, [128, 2 * DOUT], f16, "Wdec")
            bdec_sb = load_const(bdec_d, [128, DOUT], f32, "bdec")
            idxA_sb = load_const(idxA_d, [128, max(NAc, 1) * 8], i16, "idxA")
            idxB_sb = load_const(idxB_d, [128, max(NBc, 1) * 8], i16, "idxB")
            oh_sb = load_const(oh_d, [128, Gtot * D], f8, "oh")

            hA = hbuf.tile([128, 2, PERP], f16, tag="hA")
            hB = hbuf.tile([128, 2, PERP], f16, tag="hB")
            part = hbuf.tile([128, 2, PERP], f16, tag="part")

            def dense_emit(j, l, hsrc, stg):
                """stg[:, j%4, :] = (hsrc tile j @ Wg[l]) * dinv (fp8)."""
                cols = slice(j * 128, (j + 1) * 128)
                ps = psg.tile([128, HID], f32, tag="gps")
                nc.tensor.matmul(
                    ps[:],
                    hsrc[:, 0, cols],
                    Wg_sb[:, (l * 2 + 0) * HID : (l * 2 + 1) * HID],
                    start=True,
                    stop=False,
                )
                nc.tensor.matmul(
                    ps[:],
                    hsrc[:, 1, cols],
                    Wg_sb[:, (l * 2 + 1) * HID : (l * 2 + 2) * HID],
                    start=False,
                    stop=True,
                )
                nc.vector.tensor_scalar(
                    stg[:, j % 4, :],
                    ps[:],
                    dinv_sb[:, j : j + 1],
                    None,
                    mybir.AluOpType.mult,
                )

            def stg_flush(stg, j):
                tb = j - j % 4
                nb = j % 4 + 1
                rows = slice(tb * 128, (tb + nb) * 128)
                nc.sync.dma_start(
                    g_local[rows, :].rearrange("(j p) f -> p j f", p=128),
                    stg[:, 0:nb, :],
                )

            def ag_emit(gfull_pair, k):
                if "ag" in ABLATE:
                    return
                nc.gpsimd.collective_compute(
                    "AllGather",
                    mybir.AluOpType.bypass,
                    replica_groups=[core_ids],
                    ins=[g_local[CH_LR[k] : CH_LR[k + 1], :]],
                    outs=[gfull_pair[k][:]],
                )

            def decode_emit(j, hsrc, ot):
                cols = slice(j * 128, (j + 1) * 128)
                ps = psg.tile([128, DOUT], f32, tag="gps")
                nc.tensor.matmul(
                    ps[:],
                    hsrc[:, 0, cols],
                    Wdec_sb[:, 0:DOUT],
                    start=True,
                    stop=False,
                )
                nc.tensor.matmul(
                    ps[:],
                    hsrc[:, 1, cols],
                    Wdec_sb[:, DOUT : 2 * DOUT],
                    start=False,
                    stop=True,
                )
                nc.vector.tensor_tensor(
                    ot[:, j % 4, :], ps[:], bdec_sb[:], mybir.AluOpType.add
                )

            def out_flush(ot, j):
                tb = j - j % 4
                nb = j % 4 + 1
                rows = slice(tb * 128, (tb + nb) * 128)
                nc.sync.dma_start(
                    out_d[rows, :].rearrange("(j p) f -> p j f", p=128),
                    ot[:, 0:nb, :],
                )

            def alloc_gfull():
                gfa = dramgAp.tile(
                    [AROWS, HID], f8, tag="g_fullA", addr_space="Shared"
                )
                gfb = dramgBp.tile(
                    [BROWS, HID], f8, tag="g_fullB", addr_space="Shared"
                )
                return gfa, gfb

            # ---- stage 0: embedding + dense(Wg[0]) fused per tile
            g_full_next = alloc_gfull()
            stg = None
            xt = None
            for j in range(NT128):
                if j % 4 == 0:
                    stg = gstg.tile([128, 4, HID], f8, tag="gstg")
                    xt = xstg.tile([DIN, 4 * 128], f16, tag="xstg")
                    nxc = min(4 * 128, PERP - j * 128)
                    nc.sync.dma_start(
                        xt[:, 0:nxc], xT_d[:, j * 128 : j * 128 + nxc]
                    )
                xcols = slice((j % 4) * 128, (j % 4 + 1) * 128)
                cols = slice(j * 128, (j + 1) * 128)
                pse0 = psg.tile([128, 128], f32, tag="gps")
                nc.tensor.matmul(
                    pse0[:], Wemb_sb[:, 0:128], xt[:, xcols], start=True, stop=True
                )
                nc.scalar.activation(
                    hA[:, 0, cols], pse0[:], Relu, bias=bemb_sb[:, 0:1], scale=1.0
                )
                pse1 = psg.tile([128, 128], f32, tag="gps")
                nc.tensor.matmul(
                    pse1[:], Wemb_sb[:, 128:256], xt[:, xcols], start=True, stop=True
                )
                nc.scalar.activation(
                    hA[:, 1, cols], pse1[:], Relu, bias=bemb_sb[:, 1:2], scale=1.0
                )
                if "gphase" not in ABLATE:
                    dense_emit(j, 0, hA, stg)
                    if j % 4 == 3 or j == NT128 - 1:
                        stg_flush(stg, j)
                if j in AGJ:
                    ag_emit(g_full_next, AGJ[j])

            # ---- rounds: scatter(table r) + dense(Wg[r+1]) or decode
            hcur, hnext = hA, hB
            for r in range(N_LAYERS):
                g_full = g_full_next
                last = r == N_LAYERS - 1
                if not last:
                    g_full_next = alloc_gfull()

                callA = {}
                callB = {}
                qsel = [0]  # global round-robin over 4 SWDGE queues

                def ensure_call(k, calls, pool, tag, src_ap, idx_sb, nstream, qi):
                    if k in calls:
                        return calls[k]
                    nch = min(MAXCH, nstream - k * MAXCH)
                    mt = pool.tile([128, MAXCH, HID], f8, tag=tag)
                    if "gather" in ABLATE:
                        nc.vector.memset(mt[:, 0:1, 0:16], 0.0)
                        calls[k] = mt
                        return mt
                    nc.gpsimd.dma_gather(
                        mt[:, 0:nch, :],
                        src_ap,
                        idx_sb[:, k * MAXCH * 8 : (k * MAXCH + nch) * 8],
                        nch * 128,
                        nch * 128,
                        HID,
                        single_packet=False,
                        queue_num=qsel[0],
                    )
                    qsel[0] = (qsel[0] + 1) % 4
                    calls[k] = mt
                    return mt

                def group_mm(dt_, gi, ps0, ps1, st_, sp):
                    g = int(gb[dt_]) + gi
                    ohcol = g * D
                    if gi < cA[dt_]:
                        ch = int(AoffC[dt_]) + gi
                        mt = ensure_call(
                            ch // MAXCH, callA, msgAp, "msgA",
                            g_full[0][:], idxA_sb, NAc, 0,
                        )
                    else:
                        ch = int(BoffC[dt_]) + (gi - int(cA[dt_]))
                        mt = ensure_call(
                            ch // MAXCH, callB, msgBp, "msgB",
                            g_full[1][:], idxB_sb, NBc, 1,
                        )
                    c = ch % MAXCH
                    nc.tensor.matmul(
                        ps0[:],
                        mt[:, c, 0:128],
                        oh_sb[:, ohcol : ohcol + D],
                        start=st_,
                        stop=sp,
                    )
                    nc.tensor.matmul(
                        ps1[:],
                        mt[:, c, 128:256],
                        oh_sb[:, ohcol : ohcol + D],
                        start=st_,
                        stop=sp,
                    )

                # ---- pass A: A-stream partial sums -> part (fp16)
                for dt_ in range(NT64):
                    ps0 = pss.tile([128, D], f32, tag="ps0")
                    ps1 = pss.tile([128, D], f32, tag="ps1")
                    nga = int(cA[dt_])
                    for gi in range(nga):
                        if "scatter_mm" in ABLATE and gi > 0:
                            continue
                        sp = (gi == nga - 1) or "scatter_mm" in ABLATE
                        group_mm(dt_, gi, ps0, ps1, gi == 0, sp)
                    dcols = slice(dt_ * D, (dt_ + 1) * D)
                    nc.vector.tensor_scalar(
                        part[:, 0, dcols], ps0[:], 1.0, None, mybir.AluOpType.mult
                    )
                    nc.vector.tensor_scalar(
                        part[:, 1, dcols], ps1[:], 1.0, None, mybir.AluOpType.mult
                    )

                # ---- pass B: B-stream + partial + bias + relu -> hnext;
                #      dense/decode interleaved per 128-node tile
                ot = None
                for j in range(NT128):
                    if not last and j % 4 == 0:
                        stg = gstg.tile([128, 4, HID], f8, tag="gstg")
                    if last and j % 4 == 0:
                        ot = ostg.tile([128, 4, DOUT], f32, tag="ostg")
                    for dt_ in (2 * j, 2 * j + 1):
                        ps0 = pss.tile([128, D], f32, tag="ps0")
                        ps1 = pss.tile([128, D], f32, tag="ps1")
                        nga = int(cA[dt_])
                        ngb = int(cB[dt_])
                        for gi in range(ngb):
                            if "scatter_mm" in ABLATE and gi > 0:
                                continue
                            sp = (gi == ngb - 1) or "scatter_mm" in ABLATE
                            group_mm(dt_, nga + gi, ps0, ps1, gi == 0, sp)
                        dcols = slice(dt_ * D, (dt_ + 1) * D)
                        nc.vector.tensor_tensor(
                            ps0[:], ps0[:], part[:, 0, dcols], mybir.AluOpType.add
                        )
                        nc.vector.tensor_tensor(
                            ps1[:], ps1[:], part[:, 1, dcols], mybir.AluOpType.add
                        )
                        nc.scalar.activation(
                            hnext[:, 0, dcols],
                            ps0[:],
                            Relu,
                            bias=bg_sb[:, r * 2 : r * 2 + 1],
                            scale=1.0,
                        )
                        nc.scalar.activation(
                            hnext[:, 1, dcols],
                            ps1[:],
                            Relu,
                            bias=bg_sb[:, r * 2 + 1 : r * 2 + 2],
                            scale=1.0,
                        )
                    if not last:
                        if "gphase" not in ABLATE:
                            dense_emit(j, r + 1, hnext, stg)
                            if j % 4 == 3 or j == NT128 - 1:
                                stg_flush(stg, j)
                        if j in AGJ:
                            ag_emit(g_full_next, AGJ[j])
                    else:
                        decode_emit(j, hnext, ot)
                        if j % 4 == 3 or j == NT128 - 1:
                            out_flush(ot, j)
                hcur, hnext = hnext, hcur

    nc.compile()
    return nc


# ------------------------------------------------------------ entry point
def _make_in_maps(st, inputs):
    Wemb_p, bemb_p, Wg_p, bg_p, Wdec_p, bdec_p = _pack_weights(
        np.asarray(inputs["W_emb"]),
        np.asarray(inputs["b_emb"]),
        np.asarray(inputs["Wg"]),
        np.asarray(inputs["bg"]),
        np.asarray(inputs["W_dec"]),
        np.asarray(inputs["b_dec"]),
    )
    xT = _prep_x(np.asarray(inputs["x"]))
    in_maps = []
    for c in range(NC):
        in_maps.append(
            {
                "xT": xT[c],
                "dinv": st["dinv_sb"][c],
                "Wemb": Wemb_p,
                "bemb": bemb_p,
                "Wg": Wg_p,
                "bg": bg_p,
                "Wdec": Wdec_p,
                "bdec": bdec_p,
                "idxA": st["idxA_w"][c],
                "idxB": st["idxB_w"][c],
                "oh": st["oh"][c],
            }
        )
    return in_maps


def kernel(x, edge_index, edge_weights, W_emb, b_emb, Wg, bg, W_dec, b_dec):
    from concourse.bass_utils import run_bass_kernel_spmd

    st = _prep(np.asarray(edge_index), np.asarray(edge_weights))
    in_maps = _make_in_maps(
        st,
        dict(x=x, W_emb=W_emb, b_emb=b_emb, Wg=Wg, bg=bg, W_dec=W_dec, b_dec=b_dec),
    )

    nc = _build(st)

    res = run_bass_kernel_spmd(nc, in_maps, list(range(NC)))
    out = np.empty((N_NODES, DOUT), F32)
    for c in range(NC):
        out[c * PER : (c + 1) * PER] = res.results[c]["out"][:PER]
    return out.reshape(N_NODES, OUT_FEAT, FH)


# revision 31
# speedup vs baseline: 1.9831x; 1.9831x over previous
"""Distributed BasicGCN kernel for one Trainium2 chip (8 NeuronCores).

Strategy (graph/data parallel, hardcoded for N=50000 nodes / E=800000 edges):
  - Nodes are partitioned contiguously across the 8 cores (6250 each, padded
    to 6272 = 49*128). Node features h live on-chip in feature-major layout
    (hT: [128 feat x 2 halves, 6272 node cols], fp16).
  - Per GCN layer, each core computes g = (h @ W) * dinv for its node shard
    (TensorE), emits it fp8 node-major to g_local, and a 4-way CHUNKED
    AllGather replicates it into a single fp8 table g_full [50176, 256].
    The dense matmuls for layer l+1's table are interleaved per-tile into
    layer l's scatter loop, so the chunked AllGathers overlap with the
    gather/scatter stream of the previous layer.
  - Edges are partitioned by destination. The weighted scatter-sum
    agg[d] = sum_e w_e * g[src_e] is computed as one-hot matmuls: gather 128
    source rows (SWDGE dma_gather, 4 queues) into an SBUF tile [128 edges,
    256 feat] fp8, then PE-matmul with a host-precomputed one-hot weight
    matrix [128 edges, 64 dst] (fp8, SBUF-resident for all layers)
    accumulating into PSUM [128 feat, 64 dst].
  - int16 gather indices cap a stream at 32768 rows, so edges are split into
    stream A (table rows [0, 32768)) and stream B (rows [17408, 50176));
    sources in the overlap are assigned to balance the two streams. A-calls
    go to SWDGE queues 0/1, B-calls to 2/3 (no head-of-line blocking while
    the last AllGather chunk, which only stream B needs, is in flight).
  - Self-loops are folded in as ordinary edges with weight dinv[d]; the
    symmetric norm dinv[s]*ew*dinv[d] is split as (g absorbs dinv[s],
    one-hot weight absorbs ew*dinv[d]).
  - Embedding and decode layers are plain sharded matmuls (fp16).

All host-side preprocessing (degree/norm computation, edge binning, one-hot
construction) is numpy; the device program structure is identical across
cores (required by SPMD), with per-core data shipped via in_maps.
"""

import sys

sys.path.insert(0, "/opt/trn_rl_repo")

import numpy as np
from ml_dtypes import float8_e4m3

# ---------------------------------------------------------------- constants
NC = 8
N_NODES = 50000
IN_FEAT = 7
INPUT_SIZE = 12
DIN = IN_FEAT * INPUT_SIZE  # 84
HID = 256
OUT_FEAT = 7
FH = 24
DOUT = OUT_FEAT * FH  # 168
N_LAYERS = 5

PER = N_NODES // NC  # 6250 real nodes per core
NT128 = 49  # node tiles of 128 per core
PERP = NT128 * 128  # 6272 padded nodes per core
D = 64  # destination-tile size for the scatter matmul
NT64 = PERP // D  # 98 dst tiles per core

# One AllGather per layer into a single table [50176, 256] (collectives have
# ~70us fixed serial cost on the gpsimd engine, so fewer+bigger wins); int16
# gather indices cap a stream at 32768 rows: stream A reads table rows
# [0, 32768), B reads [17408, 50176); sources in the overlap are assigned
# to balance the streams.
TROWS = PERP * NC  # 50176 table rows
BBASE = 17408  # stream B's base row
AROWS = 32768
BROWS = TROWS - BBASE  # 32768

MAXCH = 16  # 128-edge chunks per dma_gather call

ABLATE = set()  # dev-only: {"gather", "scatter_mm", "ag"}

F16 = np.float16
F32 = np.float32
F8 = float8_e4m3


def _cdiv(a, b):
    return -(-a // b)


# ------------------------------------------------------------ host prep
def _prep(edge_index, edge_weights):
    """Bin edges by (core, dst-tile, gather-stream), build per-core index and
    one-hot tensors plus the (uniform) program structure."""
    src = np.asarray(edge_index[0], dtype=np.int64)
    dst = np.asarray(edge_index[1], dtype=np.int64)
    ew = np.asarray(edge_weights, dtype=F32)

    deg = np.bincount(dst, weights=ew.astype(np.float64), minlength=N_NODES).astype(
        F32
    ) + F32(1.0)
    dinv = (1.0 / np.sqrt(deg)).astype(F32)

    # edges + self loops; one-hot weight = ew * dinv[dst] (self: dinv[d])
    allsrc = np.concatenate([src, np.arange(N_NODES, dtype=np.int64)])
    alldst = np.concatenate([dst, np.arange(N_NODES, dtype=np.int64)])
    allw = np.concatenate([ew * dinv[dst], dinv]).astype(F32)

    core_d = alldst // PER
    dl = alldst % PER
    t98 = dl // D
    gt = core_d * NT64 + t98  # global tile id
    dloc = dl % D

    # source -> table row (AllGather concatenates per-core slabs)
    core_s = allsrc // PER
    sl = allsrc % PER
    grow = core_s * PERP + sl
    arow = grow  # valid iff grow < AROWS
    brow = grow - BBASE  # valid iff grow >= BBASE
    cls = np.where(grow < BBASE, 0, np.where(grow < AROWS, 1, 2)).astype(np.int64)

    NT = NC * NT64
    cnt = np.bincount(gt * 3 + cls, minlength=NT * 3).reshape(NC, NT64, 3)
    nAf, nfl, nBf = cnt[..., 0], cnt[..., 1], cnt[..., 2]
    ntot = nAf + nfl + nBf
    # per-tile chunk counts (uniform across cores); balance A/B globally
    ctot = _cdiv(ntot, 128).max(0)
    SAmin = _cdiv(nAf, 128).max(0)
    SBmin = _cdiv(nBf, 128).max(0)
    ctot = np.maximum(ctot, SAmin + SBmin)
    cA = np.clip(ctot // 2, SAmin, ctot - SBmin)
    cB = ctot - cA
    SA = cA * 128
    nA = np.minimum(nAf + nfl, SA[None, :])  # per (core, tile) edges routed to A
    nB = ntot - nA
    assert (nB <= cB[None, :] * 128).all()

    AoffC = np.concatenate([[0], np.cumsum(cA)])  # chunk offset of tile t in A stream
    BoffC = np.concatenate([[0], np.cumsum(cB)])
    NAc = int(AoffC[-1])
    NBc = int(BoffC[-1])
    gb = np.concatenate([[0], np.cumsum(cA + cB)])  # group base per tile
    Gtot = int(gb[-1])

    # order edges by (tile, class); random tertiary key (sorted sources
    # hotspot the DMA engines on gather); A-selected = first nA of each seg
    shkey = np.random.default_rng(12345).permutation(len(allsrc))
    order = np.lexsort((shkey, cls, gt))
    gts = gt[order]
    seg_start = np.searchsorted(gts, np.arange(NT))
    r = np.arange(len(order)) - seg_start[gts]  # rank within tile segment
    oc = gts // NT64
    ot = gts % NT64
    isA = r < nA[oc, ot]
    rb = r - nA[oc, ot]

    osrcA = arow[order]
    osrcB = brow[order]
    ow = allw[order]
    odloc = dloc[order]

    # stream positions
    posA = AoffC[ot] * 128 + r  # valid where isA
    posB = BoffC[ot] * 128 + rb  # valid where ~isA
    grp = np.where(isA, gb[ot] + r // 128, gb[ot] + cA[ot] + rb // 128)
    pos128 = np.where(isA, r % 128, rb % 128)

    idxA = np.zeros((NC, max(NAc, 1) * 128), np.int16)
    idxB = np.zeros((NC, max(NBc, 1) * 128), np.int16)
    a = isA
    b = ~isA
    idxA[oc[a], posA[a]] = osrcA[a].astype(np.int16)
    idxB[oc[b], posB[b]] = osrcB[b].astype(np.int16)

    oh = np.zeros((NC, 128, Gtot * D), F8)
    oh[oc, pos128, grp * D + odloc] = ow.astype(F8)

    # sanity: all used indices in range
    assert osrcA[a].max(initial=0) < AROWS and osrcA[a].min(initial=0) >= 0
    assert osrcB[b].max(initial=0) < BROWS and osrcB[b].min(initial=0) >= 0

    def wrap(idx, nchunks):
        # [NC, n*128] -> [NC, 128, n*8]: position i lives at (i%16 + 16k, i//16)
        w16 = idx.reshape(NC, nchunks * 8, 16).transpose(0, 2, 1)
        return np.ascontiguousarray(np.tile(w16, (1, 8, 1)))

    idxA_w = wrap(idxA, max(NAc, 1))
    idxB_w = wrap(idxB, max(NBc, 1))

    # per-core dinv, padded
    dinv_pad = np.zeros((NC, PERP), F32)
    dinv_pad[:, :PER] = dinv.reshape(NC, PER)
    dinv_sb = np.ascontiguousarray(dinv_pad.reshape(NC, NT128, 128).transpose(0, 2, 1))

    return dict(
        dinv=dinv,
        cA=cA,
        cB=cB,
        AoffC=AoffC,
        BoffC=BoffC,
        NAc=NAc,
        NBc=NBc,
        gb=gb,
        Gtot=Gtot,
        idxA_w=idxA_w,
        idxB_w=idxB_w,
        oh=oh,
        dinv_sb=dinv_sb,
        idxA_flat=idxA,
        idxB_flat=idxB,
    )


def _pack_weights(W_emb, b_emb, Wg, bg, W_dec, b_dec):
    """Pack weight matrices into the on-device layouts (shared across cores)."""
    Wemb_p = np.ascontiguousarray(W_emb.astype(F16))  # [84, 256]
    bemb_p = np.ascontiguousarray(
        b_emb.astype(F32).reshape(2, 128).T
    )  # [128, 2] (col h = bias[h*128+p])
    # Wg packed [128, 5*2*256]: layer l slab k -> cols [(l*2+k)*256, ...)
    Wg_p = np.zeros((128, N_LAYERS * 2 * HID), F16)
    for l in range(N_LAYERS):
        for k in range(2):
            Wg_p[:, (l * 2 + k) * HID : (l * 2 + k + 1) * HID] = Wg[l][
                k * 128 : (k + 1) * 128, :
            ].astype(F16)
    bg_p = np.ascontiguousarray(
        bg.astype(F32).reshape(N_LAYERS, 2, 128).transpose(2, 0, 1).reshape(128, -1)
    )  # [128, 5*2]: col l*2+h
    Wdec_p = np.zeros((128, 2 * DOUT), F16)
    for k in range(2):
        Wdec_p[:, k * DOUT : (k + 1) * DOUT] = W_dec[k * 128 : (k + 1) * 128, :].astype(
            F16
        )
    bdec_p = np.ascontiguousarray(np.broadcast_to(b_dec.astype(F32), (128, DOUT)))
    return Wemb_p, bemb_p, Wg_p, bg_p, Wdec_p, bdec_p


def _prep_x(x):
    """x [50000, 7, 12] -> per-core xT [84, 6272] fp16 (feature-major)."""
    xf = x.reshape(N_NODES, DIN).astype(F16)
    xT = np.zeros((NC, DIN, PERP), F16)
    for c in range(NC):
        xT[c, :, :PER] = xf[c * PER : (c + 1) * PER].T
    return xT


# ------------------------------------------------------------ device build
def _build(st):
    import concourse.bass as bass
    import concourse.mybir as mybir
    from concourse import tile, bacc

    f16 = mybir.dt.float16
    f32 = mybir.dt.float32
    f8 = mybir.dt.float8e4
    i16 = mybir.dt.int16
    Relu = mybir.ActivationFunctionType.Relu
    core_ids = list(range(NC))

    cA, cB = st["cA"], st["cB"]
    AoffC, BoffC = st["AoffC"], st["BoffC"]
    NAc, NBc, gb = st["NAc"], st["NBc"], st["gb"]
    Gtot = st["Gtot"]

    nc = bacc.Bacc(
        "TRN2",
        target_bir_lowering=False,
        debug=False,
        num_devices=NC,
        num_swdge_queues=4,
    )

    xT_d = nc.dram_tensor("xT", [DIN, PERP], f16, kind="ExternalInput")
    dinv_d = nc.dram_tensor("dinv", [128, NT128], f32, kind="ExternalInput")
    Wemb_d = nc.dram_tensor("Wemb", [DIN, HID], f16, kind="ExternalInput")
    bemb_d = nc.dram_tensor("bemb", [128, 2], f32, kind="ExternalInput")
    Wg_d = nc.dram_tensor("Wg", [128, N_LAYERS * 2 * HID], f16, kind="ExternalInput")
    bg_d = nc.dram_tensor("bg", [128, N_LAYERS * 2], f32, kind="ExternalInput")
    Wdec_d = nc.dram_tensor("Wdec", [128, 2 * DOUT], f16, kind="ExternalInput")
    bdec_d = nc.dram_tensor("bdec", [128, DOUT], f32, kind="ExternalInput")
    idxA_d = nc.dram_tensor("idxA", [128, max(NAc, 1) * 8], i16, kind="ExternalInput")
    idxB_d = nc.dram_tensor("idxB", [128, max(NBc, 1) * 8], i16, kind="ExternalInput")
    oh_d = nc.dram_tensor("oh", [128, Gtot * D], f8, kind="ExternalInput")
    out_d = nc.dram_tensor("out", [PERP, DOUT], f32, kind="ExternalOutput")

    with tile.TileContext(nc) as tc:
        with (
            tc.tile_pool(name="const", bufs=1) as constp,
            tc.tile_pool(name="hbuf", bufs=1) as hbuf,
            tc.tile_pool(name="msgA", bufs=6) as msgAp,
            tc.tile_pool(name="msgB", bufs=6) as msgBp,
            tc.tile_pool(name="gstg", bufs=2) as gstg,
            tc.tile_pool(name="ostg", bufs=2) as ostg,
            tc.tile_pool(name="psg", bufs=2, space="PSUM") as psg,
            tc.tile_pool(name="pss", bufs=3, space="PSUM") as pss,
            tc.tile_pool(name="dram", bufs=1, space="DRAM") as dramp,
            tc.tile_pool(name="dramgA", bufs=2, space="DRAM") as dramgAp,
        ):
            # ---- persistent DRAM tiles
            g_local = dramp.tile([PERP, HID], f8, tag="g_local")

            # ---- constants into SBUF
            def load_const(dram_t, shape, dtype, tag):
                t = constp.tile(shape, dtype, tag=tag)
                nc.sync.dma_start(t[:], dram_t[:])
                return t

            xT_sb = load_const(xT_d, [DIN, PERP], f16, "xT")
            dinv_sb = load_const(dinv_d, [128, NT128], f32, "dinv")
            Wemb_sb = load_const(Wemb_d, [DIN, HID], f16, "Wemb")
            bemb_sb = load_const(bemb_d, [128, 2], f32, "bemb")
            Wg_sb = load_const(Wg_d, [128, N_LAYERS * 2 * HID], f16, "Wg")
            bg_sb = load_const(bg_d, [128, N_LAYERS * 2], f32, "bg")
            Wdec_sb = load_const(Wdec_d, [128, 2 * DOUT], f16, "Wdec")
            bdec_sb = load_const(bdec_d, [128, DOUT], f32, "bdec")
            idxA_sb = load_const(idxA_d, [128, max(NAc, 1) * 8], i16, "idxA")
            idxB_sb = load_const(idxB_d, [128, max(NBc, 1) * 8], i16, "idxB")
            oh_sb = load_const(oh_d, [128, Gtot * D], f8, "oh")

            hA = hbuf.tile([128, 2, PERP], f16, tag="hA")
            hB = hbuf.tile([128, 2, PERP], f16, tag="hB")

            def dense_emit(j, l, hsrc, stg):
                """stg[:, j%4, :] = (hsrc tile j @ Wg[l]) * dinv (fp8)."""
                cols = slice(j * 128, (j + 1) * 128)
                ps = psg.tile([128, HID], f32, tag="gps")
                nc.tensor.matmul(
                    ps[:],
                    hsrc[:, 0, cols],
                    Wg_sb[:, (l * 2 + 0) * HID : (l * 2 + 1) * HID],
                    start=True,
                    stop=False,
                )
                nc.tensor.matmul(
                    ps[:],
                    hsrc[:, 1, cols],
                    Wg_sb[:, (l * 2 + 1) * HID : (l * 2 + 2) * HID],
                    start=False,
                    stop=True,
                )
                nc.vector.tensor_scalar(
                    stg[:, j % 4, :],
                    ps[:],
                    dinv_sb[:, j : j + 1],
                    None,
                    mybir.AluOpType.mult,
                )

            def stg_flush(stg, j):
                tb = j - j % 4
                nb = j % 4 + 1
                rows = slice(tb * 128, (tb + nb) * 128)
                nc.sync.dma_start(
                    g_local[rows, :].rearrange("(j p) f -> p j f", p=128),
                    stg[:, 0:nb, :],
                )

            def ag_emit(gfull):
                if "ag" in ABLATE:
                    return
                nc.gpsimd.collective_compute(
                    "AllGather",
                    mybir.AluOpType.bypass,
                    replica_groups=[core_ids],
                    ins=[g_local[:]],
                    outs=[gfull[:]],
                )

            def decode_emit(j, hsrc, ot):
                cols = slice(j * 128, (j + 1) * 128)
                ps = psg.tile([128, DOUT], f32, tag="gps")
                nc.tensor.matmul(
                    ps[:],
                    hsrc[:, 0, cols],
                    Wdec_sb[:, 0:DOUT],
                    start=True,
                    stop=False,
                )
                nc.tensor.matmul(
                    ps[:],
                    hsrc[:, 1, cols],
                    Wdec_sb[:, DOUT : 2 * DOUT],
                    start=False,
                    stop=True,
                )
                nc.vector.tensor_tensor(
                    ot[:, j % 4, :], ps[:], bdec_sb[:], mybir.AluOpType.add
                )

            def out_flush(ot, j):
                tb = j - j % 4
                nb = j % 4 + 1
                rows = slice(tb * 128, (tb + nb) * 128)
                nc.sync.dma_start(
                    out_d[rows, :].rearrange("(j p) f -> p j f", p=128),
                    ot[:, 0:nb, :],
                )

            def alloc_gfull():
                gf = dramgAp.tile(
                    [TROWS, HID], f8, tag="g_full", addr_space="Shared"
                )
                return gf

            # ---- stage 0: embedding + dense(Wg[0]) fused per tile
            g_full_next = alloc_gfull()
            stg = None
            for j in range(NT128):
                if j % 4 == 0:
                    stg = gstg.tile([128, 4, HID], f8, tag="gstg")
                cols = slice(j * 128, (j + 1) * 128)
                pse0 = psg.tile([128, 128], f32, tag="gps")
                nc.tensor.matmul(
                    pse0[:], Wemb_sb[:, 0:128], xT_sb[:, cols], start=True, stop=True
                )
                nc.scalar.activation(
                    hA[:, 0, cols], pse0[:], Relu, bias=bemb_sb[:, 0:1], scale=1.0
                )
                pse1 = psg.tile([128, 128], f32, tag="gps")
                nc.tensor.matmul(
                    pse1[:], Wemb_sb[:, 128:256], xT_sb[:, cols], start=True, stop=True
                )
                nc.scalar.activation(
                    hA[:, 1, cols], pse1[:], Relu, bias=bemb_sb[:, 1:2], scale=1.0
                )
                if "gphase" not in ABLATE:
                    dense_emit(j, 0, hA, stg)
                    if j % 4 == 3 or j == NT128 - 1:
                        stg_flush(stg, j)
            ag_emit(g_full_next)

            # ---- rounds: scatter(table r) + dense(Wg[r+1]) or decode
            hcur, hnext = hA, hB
            for r in range(N_LAYERS):
                g_full = g_full_next
                last = r == N_LAYERS - 1
                if not last:
                    g_full_next = alloc_gfull()

                callA = {}
                callB = {}
                qsel = [0]  # global round-robin over 4 SWDGE queues

                def ensure_call(k, calls, pool, tag, src_ap, idx_sb, nstream, qi):
                    if k in calls:
                        return calls[k]
                    nch = min(MAXCH, nstream - k * MAXCH)
                    mt = pool.tile([128, MAXCH, HID], f8, tag=tag)
                    if "gather" in ABLATE:
                        nc.vector.memset(mt[:, 0:1, 0:16], 0.0)
                        calls[k] = mt
                        return mt
                    nc.gpsimd.dma_gather(
                        mt[:, 0:nch, :],
                        src_ap,
                        idx_sb[:, k * MAXCH * 8 : (k * MAXCH + nch) * 8],
                        nch * 128,
                        nch * 128,
                        HID,
                        single_packet=False,
                        queue_num=qsel[0],
                    )
                    qsel[0] = (qsel[0] + 1) % 4
                    calls[k] = mt
                    return mt

                ot = None
                for j in range(NT128):
                    if not last and j % 4 == 0:
                        stg = gstg.tile([128, 4, HID], f8, tag="gstg")
                    if last and j % 4 == 0:
                        ot = ostg.tile([128, 4, DOUT], f32, tag="ostg")
                    for dt_ in (2 * j, 2 * j + 1):
                        ps0 = pss.tile([128, D], f32, tag="ps0")
                        ps1 = pss.tile([128, D], f32, tag="ps1")
                        ng = int(cA[dt_] + cB[dt_])
                        for gi in range(ng):
                            g = int(gb[dt_]) + gi
                            ohcol = g * D
                            if gi < cA[dt_]:
                                ch = int(AoffC[dt_]) + gi
                                mt = ensure_call(
                                    ch // MAXCH,
                                    callA,
                                    msgAp,
                                    "msgA",
                                    g_full[0:AROWS, :],
                                    idxA_sb,
                                    NAc,
                                    0,
                                )
                                c = ch % MAXCH
                            else:
                                ch = int(BoffC[dt_]) + (gi - int(cA[dt_]))
                                mt = ensure_call(
                                    ch // MAXCH,
                                    callB,
                                    msgBp,
                                    "msgB",
                                    g_full[BBASE:TROWS, :],
                                    idxB_sb,
                                    NBc,
                                    1,
                                )
                                c = ch % MAXCH
                            st_, sp = (gi == 0), (gi == ng - 1)
                            if "scatter_mm" in ABLATE:
                                if gi > 0:
                                    continue
                                sp = True
                            nc.tensor.matmul(
                                ps0[:],
                                mt[:, c, 0:128],
                                oh_sb[:, ohcol : ohcol + D],
                                start=st_,
                                stop=sp,
                            )
                            nc.tensor.matmul(
                                ps1[:],
                                mt[:, c, 128:256],
                                oh_sb[:, ohcol : ohcol + D],
                                start=st_,
                                stop=sp,
                            )
                        dcols = slice(dt_ * D, (dt_ + 1) * D)
                        nc.scalar.activation(
                            hnext[:, 0, dcols],
                            ps0[:],
                            Relu,
                            bias=bg_sb[:, r * 2 : r * 2 + 1],
                            scale=1.0,
                        )
                        nc.scalar.activation(
                            hnext[:, 1, dcols],
                            ps1[:],
                            Relu,
                            bias=bg_sb[:, r * 2 + 1 : r * 2 + 2],
                            scale=1.0,
                        )
                    if not last:
                        if "gphase" not in ABLATE:
                            dense_emit(j, r + 1, hnext, stg)
                            if j % 4 == 3 or j == NT128 - 1:
                                stg_flush(stg, j)
                    else:
                        decode_emit(j, hnext, ot)
                        if j % 4 == 3 or j == NT128 - 1:
                            out_flush(ot, j)
                if not last:
                    ag_emit(g_full_next)
                hcur, hnext = hnext, hcur

    nc.compile()
    return nc


# ------------------------------------------------------------ entry point
def _make_in_maps(st, inputs):
    Wemb_p, bemb_p, Wg_p, bg_p, Wdec_p, bdec_p = _pack_weights(
        np.asarray(inputs["W_emb"]),
        np.asarray(inputs["b_emb"]),
        np.asarray(inputs["Wg"]),
        np.asarray(inputs["bg"]),
        np.asarray(inputs["W_dec"]),
        np.asarray(inputs["b_dec"]),
    )
    xT = _prep_x(np.asarray(inputs["x"]))
    in_maps = []
    for c in range(NC):
        in_maps.append(
            {
                "xT": xT[c],
                "dinv": st["dinv_sb"][c],
                "Wemb": Wemb_p,
                "bemb": bemb_p,
                "Wg": Wg_p,
                "bg": bg_p,
                "Wdec": Wdec_p,
                "bdec": bdec_p,
                "idxA": st["idxA_w"][c],
                "idxB": st["idxB_w"][c],
                "oh": st["oh"][c],
            }
        )
    return in_maps


def kernel(x, edge_index, edge_weights, W_emb, b_emb, Wg, bg, W_dec, b_dec):
    from concourse.bass_utils import run_bass_kernel_spmd

    st = _prep(np.asarray(edge_index), np.asarray(edge_weights))
    in_maps = _make_in_maps(
        st,
        dict(x=x, W_emb=W_emb, b_emb=b_emb, Wg=Wg, bg=bg, W_dec=W_dec, b_dec=b_dec),
    )

    nc = _build(st)

    res = run_bass_kernel_spmd(nc, in_maps, list(range(NC)))
    out = np.empty((N_NODES, DOUT), F32)
    for c in range(NC):
        out[c * PER : (c + 1) * PER] = res.results[c]["out"][:PER]
    return out.reshape(N_NODES, OUT_FEAT, FH)
